# revision 3
# baseline (speedup 1.0000x reference)
"""CT self-attention (causal + 2 future frames) for Trainium2, 8 NeuronCores.

Sharding: batch (4-way) x head-group (2-way): core c = 2*b + g handles batch b,
heads [8g, 8g+8). Each core computes its QKV projection slice, banded
attention for its 8 heads, and a partial output projection; the host sums the
two partial outputs per batch and adds the (host-folded) biases.

All matmuls run in bf16 (1 col/cycle @ 2.4 GHz warm) with fp32 PSUM.
Schedule is built for PE density (HAM stays at full clock):
  - startup: f-major sweep accumulating V t0-4 + K0 (first 2 blocks) + Q0
    (block 0) in 8 PSUM banks while the x^T / w DMAs stream in (4 DMA
    queues), then the remaining K0/Q0 blocks
  - attention for block q5=0 interleaves the rest of the QKV projection
    (V t5-15, K1-3, Q1-3) between head pairs as PE gap filler
  - attention: scores S_T = K^T-tile.T @ Q (2 heads row-tiled), exp on
    ScalarE (scale 1/8, no bias), CT mask applied post-exp as a DVE 0/1
    multiply on the diagonal-tile window, AV with a ones column on V
    (denominator on partition 64); key-padding is folded into V rows and
    the ones column (both zeroed for padded keys), not the exp bias
  - normalize per head pair: 2 denominator rows -> [2,512] fast reciprocal
    (fp32), one [2,128] fp32 selector matmul broadcasts both heads'
    reciprocals across 128 partitions, 2 DVE mults into AT; deferred into
    the next block's stream (immediate for the last block)
  - output projection per 128-query tile once all 4 pairs are normalized
"""
import math
from contextlib import ExitStack

import numpy as np

B, T, D, H = 4, 2048, 1024, 16
HD = D // H            # 64
L = 2                  # max_future_frames
NCORES = 8
HPG = 8                # heads per group/core
NPAIR = 4              # head pairs per core
FCH = 8                # feature chunks (D / 128)
TQ5 = 4                # 512-wide query tiles
NKT = 16               # 128-wide key tiles

_BUILT = {}


def _build_nc():
    import concourse.tile as tile
    from concourse import bacc, mybir

    dt = mybir.dt
    f32, bf16 = dt.float32, dt.bfloat16
    Exp = mybir.ActivationFunctionType.Exp
    MUL = mybir.AluOpType.mult
    ADD = mybir.AluOpType.add

    nc = bacc.Bacc(None, target_bir_lowering=False)
    xT_d = nc.dram_tensor("xT", [FCH, 128, T], bf16, kind="ExternalInput")
    wqkvT_d = nc.dram_tensor("wqkvT", [FCH, 128, 3 * 512], bf16, kind="ExternalInput")
    woutT_d = nc.dram_tensor("woutT", [NPAIR, 128, D], bf16, kind="ExternalInput")
    bq_d = nc.dram_tensor("bq", [128, NPAIR], f32, kind="ExternalInput")
    bk_d = nc.dram_tensor("bk", [128, NPAIR], f32, kind="ExternalInput")
    kp_d = nc.dram_tensor("kp01", [128, NKT], f32, kind="ExternalInput")
    vm_d = nc.dram_tensor("vm", [128, 5, 2, 128], bf16, kind="ExternalInput")
    sel2_d = nc.dram_tensor("sel2", [2, 128], f32, kind="ExternalInput")
    vones_d = nc.dram_tensor("vones", [128, NKT * HPG], bf16, kind="ExternalInput")
    out_d = nc.dram_tensor("out_part", [T, D], f32, kind="ExternalOutput")

    # per-off masked-window geometry for the 5 diagonal key tiles
    def mwin(off):
        q0 = max(0, 128 * off - L)
        m1 = min(512, 128 * off + 126)
        return q0, m1

    with tile.TileContext(nc) as tc, \
         nc.allow_low_precision(reason="bf16 matmul fast path"), \
         ExitStack() as top:
        pers = top.enter_context(tc.tile_pool(name="pers", bufs=1))
        xT_sb = pers.tile([128, FCH, T], bf16, name="xT_sb")
        wk_sb = pers.tile([128, FCH, 3 * 512], bf16, name="wk_sb")
        QT = pers.tile([128, NPAIR, T], bf16, name="QT")
        KT = pers.tile([128, NPAIR, T], bf16, name="KT")
        Vt = pers.tile([128, NKT, HPG, HD + 1], bf16, name="Vt")
        AT = pers.tile([128, NPAIR, T], bf16, name="AT")
        wo_sb = pers.tile([128, NPAIR, D], bf16, name="wo_sb")
        vm_sb = pers.tile([128, 5, 2, 128], bf16, name="vm_sb")
        kp_sb = pers.tile([128, NKT], f32, name="kp_sb")
        bq_sb = pers.tile([128, NPAIR], f32, name="bq_sb")
        bk_sb = pers.tile([128, NPAIR], f32, name="bk_sb")
        sel2_sb = pers.tile([2, 128], f32, name="sel2_sb")
        vones_sb = pers.tile([128, NKT * HPG], bf16, name="vones_sb")

        # priority-sliced input loads over 4 DMA queues: the f-major startup
        # sweep needs, per f, only the [Q0|K0|V] weight slice and the first
        # 1024 x columns -- stream those first in f order (each queue's FIFO
        # preserves f-major arrival), defer the remaining weight pairs
        # (fillers), the upper x columns, and w_out
        qs = [nc.sync, nc.scalar, nc.gpsimd]
        nc.scalar.dma_start(vm_sb[:], vm_d[:])
        nc.scalar.dma_start(kp_sb[:], kp_d[:])
        nc.scalar.dma_start(bq_sb[:], bq_d[:])
        nc.scalar.dma_start(bk_sb[:], bk_d[:])
        nc.scalar.dma_start(sel2_sb[:], sel2_d[:])
        nc.scalar.dma_start(vones_sb[:], vones_d[:])
        for f in range(FCH):
            qs[f % 3].dma_start(wk_sb[:, f, 0:768], wqkvT_d[f, :, 0:768])
            qs[(f + 1) % 3].dma_start(xT_sb[:, f, 0:1024], xT_d[f, :, 0:1024])
        for f in range(FCH):
            qs[f % 3].dma_start(wk_sb[:, f, 768:1536], wqkvT_d[f, :, 768:1536])
        for f in range(FCH):
            qs[(f + 1) % 3].dma_start(xT_sb[:, f, 1024:2048], xT_d[f, :, 1024:2048])
        for cchunk in range(NPAIR):
            qs[(cchunk + 2) % 3].dma_start(wo_sb[:, cchunk, :], woutT_d[cchunk])
        nc.vector.tensor_copy(
            Vt[:, :, :, HD],
            vones_sb[:].rearrange("p (a b) -> p a b", a=NKT))

        # host packs the weight columns as [Q0 | K0 | V | Q1-3 | K1-3] so the
        # startup sweep's needs are one contiguous priority DMA slice
        def wQ(f, p):
            return wk_sb[:, f, 0:128] if p == 0 else \
                wk_sb[:, f, 768 + 128 * (p - 1):768 + 128 * p]

        def wK(f, p):
            return wk_sb[:, f, 128:256] if p == 0 else \
                wk_sb[:, f, 1152 + 128 * (p - 1):1152 + 128 * p]

        def wV(f):
            return wk_sb[:, f, 256:768]

        def v_copy(pv, t):
            # key-padding folded in: padded key rows of V are zeroed (the
            # ones column comes pre-masked from the host via vones)
            nc.vector.tensor_scalar(
                Vt[:, t, :, 0:HD],
                pv[:].rearrange("p (h d) -> p h d", h=HPG),
                kp_sb[:, t:t + 1], None, MUL)

        def qk_store(pqk, tgt, t5):
            pair = tgt % 4
            dst = (QT if tgt < 4 else KT)[:, pair, t5 * 512:(t5 + 1) * 512]
            bias = (bq_sb if tgt < 4 else bk_sb)[:, pair:pair + 1]
            nc.vector.tensor_scalar(dst, pqk[:], bias, None, ADD)

        # ---- startup: f-major sweep (PE works while x^T still streams) ----
        with tc.tile_pool(name="psUp", bufs=1, space="PSUM") as psUp:
            pvs = [psUp.tile([128, 512], f32, name=f"pv{t}", tag=f"u{t}")
                   for t in range(5)]
            pk0 = psUp.tile([128, 512], f32, name="pk0", tag="u5")
            pk1 = psUp.tile([128, 512], f32, name="pk1", tag="u6")
            pq0 = psUp.tile([128, 512], f32, name="pq0", tag="u7")
            for f in range(FCH):
                st = dict(start=(f == 0), stop=(f == FCH - 1))
                for t in range(5):
                    nc.tensor.matmul(pvs[t][:], xT_sb[:, f, t * 128:(t + 1) * 128],
                                     wV(f), **st)
                nc.tensor.matmul(pk0[:], wK(f, 0), xT_sb[:, f, 0:512], **st)
                nc.tensor.matmul(pk1[:], wK(f, 0), xT_sb[:, f, 512:1024], **st)
                nc.tensor.matmul(pq0[:], wQ(f, 0), xT_sb[:, f, 0:512], **st)
            for t in range(5):
                v_copy(pvs[t], t)
            nc.vector.tensor_scalar(KT[:, 0, 0:512], pk0[:], bk_sb[:, 0:1], None, ADD)
            nc.vector.tensor_scalar(KT[:, 0, 512:1024], pk1[:], bk_sb[:, 0:1], None, ADD)
            nc.vector.tensor_scalar(QT[:, 0, 0:512], pq0[:], bq_sb[:, 0:1], None, ADD)

        # ---- attention (with projection filler inside block 0) ----
        with tc.tile_pool(name="eps", bufs=3) as epool, \
             tc.tile_pool(name="nsb", bufs=1) as nsb, \
             tc.tile_pool(name="avp", bufs=1) as avp, \
             tc.tile_pool(name="osb", bufs=2) as osb, \
             tc.tile_pool(name="psAv", bufs=1, space="PSUM") as psAv, \
             tc.tile_pool(name="psSc", bufs=2, space="PSUM") as psSc, \
             tc.tile_pool(name="psX", bufs=1, space="PSUM") as psX:

            nx = [0]

            def emitQK(tgt, t5):
                pqk = psX.tile([128, 512], f32, name="pqk", tag=f"x{nx[0] % 2}")
                nx[0] += 1
                w = wQ if tgt < 4 else wK
                for f in range(FCH):
                    nc.tensor.matmul(pqk[:], w(f, tgt % 4),
                                     xT_sb[:, f, t5 * 512:(t5 + 1) * 512],
                                     start=(f == 0), stop=(f == FCH - 1))
                qk_store(pqk, tgt, t5)

            def emitV(t):
                pv = psX.tile([128, 512], f32, name="pv", tag=f"x{nx[0] % 2}")
                nx[0] += 1
                for f in range(FCH):
                    nc.tensor.matmul(pv[:], xT_sb[:, f, t * 128:(t + 1) * 128],
                                     wV(f), start=(f == 0), stop=(f == FCH - 1))
                v_copy(pv, t)

            # remaining startup QK blocks (t5-major, 2-bank pipeline)
            rest = [(4, 2), (4, 3), (0, 1), (0, 2), (0, 3)]
            fillers = [
                [("V", t) for t in range(5, 9)] + [(5, t5) for t5 in range(4)]
                + [(1, t5) for t5 in range(4)],
                [("V", t) for t in range(9, 13)] + [(6, t5) for t5 in range(4)]
                + [(2, t5) for t5 in range(4)],
                [(7, t5) for t5 in range(4)]
                + [(3, t5) for t5 in range(4)],
                [],
            ]
            # V tiles 13-15 are only needed from block 3 on: emit them as
            # PE filler inside the ACT-bound block-2 stretch instead
            late_v = {(2, p): 13 + p for p in range(3)}
            for tgt, t5 in rest:
                emitQK(tgt, t5)

            norm_pend = {}

            def emit_normalize_pair(q5, p):
                av2, rc32 = norm_pend.pop((q5, p))
                qs_ = slice(q5 * 512, (q5 + 1) * 512)
                bc = psX.tile([128, 512], f32, name="bc", tag="x0")
                nc.tensor.matmul(bc[:], sel2_sb[:], rc32[:],
                                 start=True, stop=True)
                nc.vector.tensor_tensor(AT[0:64, p, qs_], av2[0:64, 0, :],
                                        bc[0:64, :], MUL)
                nc.vector.tensor_tensor(AT[64:128, p, qs_], av2[0:64, 1, :],
                                        bc[64:128, :], MUL)

            def emit_proj(q5):
                for tq in range(4):
                    t = 4 * q5 + tq
                    tsl = slice(t * 128, (t + 1) * 128)
                    po0 = psX.tile([128, 512], f32, name="po0", tag="x0")
                    po1 = psX.tile([128, 512], f32, name="po1", tag="x1")
                    for cchunk in range(NPAIR):
                        lhsT = AT[:, cchunk, tsl]
                        nc.tensor.matmul(po0[:], lhsT, wo_sb[:, cchunk, 0:512],
                                         start=(cchunk == 0), stop=(cchunk == 3))
                        nc.tensor.matmul(po1[:], lhsT, wo_sb[:, cchunk, 512:1024],
                                         start=(cchunk == 0), stop=(cchunk == 3))
                    ot = osb.tile([128, D], f32, name="ot", tag="ot")
                    nc.vector.tensor_copy(ot[:, 0:512], po0[:])
                    nc.vector.tensor_copy(ot[:, 512:1024], po1[:])
                    (nc.sync if t % 2 else nc.gpsimd).dma_start(
                        out_d[tsl, :], ot[:])

            # Flat software pipeline across pairs/blocks: each pair's last AV
            # is deferred past the next pair's first score group so the PE
            # never drains while ScalarE finishes the last exp.
            pend_av = [None]

            def flush_av():
                if pend_av[0] is not None:
                    pend_av[0]()
                    pend_av[0] = None

            def finish_pair(q5, p, avA, avB):
                # AV out of PSUM into one combined tile (slot-freeing copies
                # first), then both denominator rows gathered with a single
                # SBUF->SBUF DMA (DVE is partition-locked) for one batched
                # fp32 reciprocal
                av2 = avp.tile([HD + 1, 2, 512], f32,
                               name=f"av2_{p}", tag=f"avp{p}")
                nc.vector.tensor_copy(av2[:, 0, :], avA[:])
                nc.vector.tensor_copy(av2[:, 1, :], avB[:])
                d2 = nsb.tile([2, 512], f32, name="d2", tag=f"dp{p}")
                nc.gpsimd.dma_start(d2[:], av2[64:65, :, :])
                rc32 = nsb.tile([2, 512], f32, name="rc32", tag=f"di{p}")
                nc.vector.reciprocal_approx_fast(rc32[:], d2[:])
                norm_pend[(q5, p)] = (av2, rc32)

            for q5 in range(TQ5):
                nkt = min(4 * q5 + 5, NKT)
                q5s = q5 * 512
                for p in range(NPAIR):
                    avA = psAv.tile([HD + 1, 512], f32, name="avA", tag="avA")
                    avB = psAv.tile([HD + 1, 512], f32, name="avB", tag="avB")
                    for kt in range(nkt):
                        ks = slice(kt * 128, (kt + 1) * 128)
                        off = kt - 4 * q5
                        masked = off >= 0
                        # masked tiles only affect queries >= q0
                        q0 = mwin(off)[0] if masked else 0
                        qsl = slice(q5s + q0, q5s + 512)
                        sc2 = psSc.tile([128, 2, 512], f32, name="sc2", tag="sc2")
                        nc.tensor.matmul(sc2[:, 0, q0:512],
                                         KT[0:64, p, ks], QT[0:64, p, qsl],
                                         start=True, stop=True,
                                         tile_position=(0, 0))
                        nc.tensor.matmul(sc2[:, 1, q0:512],
                                         KT[64:128, p, ks], QT[64:128, p, qsl],
                                         start=True, stop=True,
                                         tile_position=(64, 0),
                                         skip_group_check=True)
                        flush_av()
                        if kt == 1:
                            # deferred bookkeeping once the pipeline is primed:
                            # block-delayed normalize of (q5-1, p), plus
                            # pair-delayed normalize inside the last block
                            if q5 >= 1 and (q5 - 1, p) in norm_pend:
                                emit_normalize_pair(q5 - 1, p)
                                if p == 3:
                                    emit_proj(q5 - 1)
                        if kt == 2 and q5 == TQ5 - 1 and p >= 1 and \
                                (q5, p - 1) in norm_pend:
                            emit_normalize_pair(q5, p - 1)
                        e2 = epool.tile([128, 2, 512], bf16, name="e2", tag="e2")
                        nc.scalar.activation(e2[:, :, q0:512], sc2[:, :, q0:512],
                                             Exp, scale=1.0 / math.sqrt(HD))
                        if masked:
                            # CT mask post-exp: zero the invalid triangle of
                            # the diagonal tile with a 0/1 DVE multiply
                            m0, m1 = mwin(off)
                            nc.vector.tensor_tensor(
                                e2[:, :, m0:m1], e2[:, :, m0:m1],
                                vm_sb[:, off, :, 0:m1 - m0], MUL)

                        def mk_av(kt=kt, e2=e2, q0=q0, avA=avA, avB=avB,
                                  p=p, nkt=nkt, q5=q5):
                            nc.tensor.matmul(avA[0:65, q0:512],
                                             Vt[:, kt, 2 * p, :],
                                             e2[:, 0, q0:512],
                                             start=(kt == 0), stop=(kt == nkt - 1),
                                             skip_group_check=True)
                            nc.tensor.matmul(avB[0:65, q0:512],
                                             Vt[:, kt, 2 * p + 1, :],
                                             e2[:, 1, q0:512],
                                             start=(kt == 0), stop=(kt == nkt - 1),
                                             skip_group_check=True)
                            if kt == nkt - 1:
                                finish_pair(q5, p, avA, avB)
                        pend_av[0] = mk_av
                    # projection filler inside block 0 keeps the PE warm
                    if q5 == 0:
                        for j, item in enumerate(fillers[p]):
                            if item[0] == "V":
                                emitV(item[1])
                            else:
                                emitQK(item[0], item[1])
                            if j == 0:
                                flush_av()
                    if (q5, p) in late_v:
                        emitV(late_v[(q5, p)])
            # drain: last pair's AV, its normalize, last projection
            flush_av()
            emit_normalize_pair(TQ5 - 1, 3)
            emit_proj(TQ5 - 1)

    nc.finalize()
    return nc


def _host_inputs(x, key_padding_mask, w_qkv, b_qkv, w_out):
    """Per-core input dicts."""
    import ml_dtypes

    f32 = np.float32
    bf = ml_dtypes.bfloat16
    # CT-mask valid multipliers for the 5 diagonal tile offsets (shared)
    vm = np.ones((128, 5, 2, 128), f32)
    k = np.arange(128)[:, None]
    for off in range(5):
        q0 = max(0, 128 * off - L)
        m1 = min(512, 128 * off + 126)
        w = m1 - q0
        j = np.arange(w)[None, :]
        valid = (128 * off + k <= q0 + j + L).astype(f32)   # [128, w]
        vm[:, off, 0, 0:w] = valid
        vm[:, off, 1, 0:w] = valid
    vm = vm.astype(bf)
    sel2 = np.zeros((2, 128), f32)
    sel2[0, 0:64] = 1.0
    sel2[1, 64:128] = 1.0

    in_maps = []
    for c in range(NCORES):
        b, g = divmod(c, 2)
        # channel rows for this group's Q/K (pairs of heads -> 128 rows each)
        qrows = np.concatenate(
            [w_qkv[64 * (8 * g + 2 * p):64 * (8 * g + 2 * p) + 128] for p in range(NPAIR)])
        krows = np.concatenate(
            [w_qkv[D + 64 * (8 * g + 2 * p):D + 64 * (8 * g + 2 * p) + 128] for p in range(NPAIR)])
        vrows = w_qkv[2 * D + 512 * g:2 * D + 512 * g + 512]
        # column order [Q0 | K0 | V | Q1-3 | K1-3]: the startup sweep's
        # weights form one contiguous priority DMA slice
        w_all = np.concatenate([qrows[0:128], krows[0:128], vrows,
                                qrows[128:512], krows[128:512]], 0)
        wqkvT = np.ascontiguousarray(w_all.T).reshape(FCH, 128, 3 * 512)
        bq = np.stack(
            [b_qkv[64 * (8 * g + 2 * p):64 * (8 * g + 2 * p) + 128] for p in range(NPAIR)], 1)
        bk = np.stack(
            [b_qkv[D + 64 * (8 * g + 2 * p):D + 64 * (8 * g + 2 * p) + 128] for p in range(NPAIR)], 1)
        xT = np.ascontiguousarray(x[b].T).reshape(FCH, 128, T)
        woutT = np.ascontiguousarray(w_out.T[512 * g:512 * g + 512]).reshape(NPAIR, 128, D)
        # key-padding as a 0/1 keep-multiplier on V rows + the ones column
        kp01 = np.ascontiguousarray(
            (1.0 - key_padding_mask[b].astype(f32)).reshape(NKT, 128).T)
        vones = np.repeat(kp01, HPG, axis=1).astype(bf)
        in_maps.append({
            "xT": xT.astype(bf), "wqkvT": wqkvT.astype(bf),
            "woutT": woutT.astype(bf),
            "bq": bq.astype(f32), "bk": bk.astype(f32),
            "kp01": kp01.astype(f32),
            "vm": vm, "sel2": sel2.astype(f32), "vones": vones,
        })
    return in_maps


def kernel(x, key_padding_mask, w_qkv, b_qkv, w_out, b_out):
    from concourse.bass_utils import run_bass_kernel_spmd

    x = np.asarray(x, np.float32)
    key_padding_mask = np.asarray(key_padding_mask)
    w_qkv = np.asarray(w_qkv, np.float32)
    b_qkv = np.asarray(b_qkv, np.float32)
    w_out = np.asarray(w_out, np.float32)
    b_out = np.asarray(b_out, np.float32)

    if "nc" not in _BUILT:
        _BUILT["nc"] = _build_nc()
    nc = _BUILT["nc"]

    in_maps = _host_inputs(x, key_padding_mask, w_qkv, b_qkv, w_out)
    res = run_bass_kernel_spmd(nc, in_maps, core_ids=list(range(NCORES)))
    out = np.empty((B, T, D), np.float32)
    for b in range(B):
        out[b] = res.results[2 * b]["out_part"] + res.results[2 * b + 1]["out_part"]
    # host-folded biases: b_out plus the V-bias pushed through the projection
    bv = b_qkv[2 * D:3 * D]
    out += (b_out + bv @ w_out.T)[None, None, :].astype(np.float32)
    return out


# revision 6
# speedup vs baseline: 1.0189x; 1.0189x over previous
"""CT self-attention (causal + 2 future frames) for Trainium2, 8 NeuronCores.

Sharding: batch (4-way) x head-group (2-way): core c = 2*b + g handles batch b,
heads [8g, 8g+8). Each core computes its QKV projection slice, banded
attention for its 8 heads, and a partial output projection; the host sums the
two partial outputs per batch and adds the (host-folded) biases.

All matmuls run in bf16 (1 col/cycle @ 2.4 GHz warm) with fp32 PSUM.
Schedule is built for PE density (HAM stays at full clock) and balances the
PE against the exp-bound ScalarE stream:
  - startup: f-major sweep accumulating V t0-4 + K(0,0-1) + Q(0,0) in 8 PSUM
    banks while the x^T / w DMAs stream in (3 DMA queues, split first
    chunks); attention starts right after the sweep
  - the rest of the QKV projection (V t5-15, all other K/Q blocks) is
    distributed as PE gap filler across ALL attention blocks, placed by a
    dependency schedule (each unit lands just before the block that consumes
    it) so the PE never drains during the exp-bound late blocks
  - attention: scores S_T = K^T-tile.T @ Q (2 heads row-tiled), CT mask via
    accumulating -1e9*I @ MQ matmul (mq holds only the 128-wide diagonal
    window), exp on ScalarE (scale 1/8, no bias), AV with a ones column on V
    (denominator on partition 64); key-padding is folded into V rows and the
    ones column (both zeroed for padded keys), not the exp bias
  - normalize per head pair: 2 denominator rows -> [2,512] fast reciprocal
    (fp32), one [2,128] fp32 selector matmul broadcasts both heads'
    reciprocals across 128 partitions, 2 DVE mults into AT; deferred into
    the next block's stream (immediate for the last block)
  - output projection per 128-query tile once all 4 pairs are normalized
"""
import math
from contextlib import ExitStack

import numpy as np

B, T, D, H = 4, 2048, 1024, 16
HD = D // H            # 64
L = 2                  # max_future_frames
NCORES = 8
HPG = 8                # heads per group/core
NPAIR = 4              # head pairs per core
FCH = 8                # feature chunks (D / 128)
TQ5 = 4                # 512-wide query tiles
NKT = 16               # 128-wide key tiles
NEG = -1.0e9

_BUILT = {}


def _mwin(off):
    q0 = max(0, 128 * off - L)
    m1 = min(512, 128 * off + 126)
    return q0, m1


def _build_nc():
    import concourse.tile as tile
    from concourse import bacc, mybir

    dt = mybir.dt
    f32, bf16 = dt.float32, dt.bfloat16
    Exp = mybir.ActivationFunctionType.Exp
    MUL = mybir.AluOpType.mult
    ADD = mybir.AluOpType.add

    nc = bacc.Bacc(None, target_bir_lowering=False)
    xT_d = nc.dram_tensor("xT", [FCH, 128, T], bf16, kind="ExternalInput")
    wqkvT_d = nc.dram_tensor("wqkvT", [FCH, 128, 3 * 512], bf16, kind="ExternalInput")
    woutT_d = nc.dram_tensor("woutT", [NPAIR, 128, D], bf16, kind="ExternalInput")
    bq_d = nc.dram_tensor("bq", [128, NPAIR], f32, kind="ExternalInput")
    bk_d = nc.dram_tensor("bk", [128, NPAIR], f32, kind="ExternalInput")
    kp_d = nc.dram_tensor("kp01", [128, NKT], f32, kind="ExternalInput")
    mq_d = nc.dram_tensor("mq", [128, 5, 128], bf16, kind="ExternalInput")
    mk_d = nc.dram_tensor("mk", [128, 128], bf16, kind="ExternalInput")
    sel2_d = nc.dram_tensor("sel2", [2, 128], f32, kind="ExternalInput")
    vones_d = nc.dram_tensor("vones", [128, NKT * HPG], bf16, kind="ExternalInput")
    out_d = nc.dram_tensor("out_part", [T, D], f32, kind="ExternalOutput")

    with tile.TileContext(nc) as tc, \
         nc.allow_low_precision(reason="bf16 matmul fast path"), \
         ExitStack() as top:
        pers = top.enter_context(tc.tile_pool(name="pers", bufs=1))
        xT_sb = pers.tile([128, FCH, T], bf16, name="xT_sb")
        wk_sb = pers.tile([128, FCH, 3 * 512], bf16, name="wk_sb")
        QT = pers.tile([128, NPAIR, T], bf16, name="QT")
        KT = pers.tile([128, NPAIR, T], bf16, name="KT")
        Vt = pers.tile([128, NKT, HPG, HD + 1], bf16, name="Vt")
        AT = pers.tile([128, NPAIR, T], bf16, name="AT")
        wo_sb = pers.tile([128, NPAIR, D], bf16, name="wo_sb")
        mq_sb = pers.tile([128, 5, 128], bf16, name="mq_sb")
        mk_sb = pers.tile([128, 128], bf16, name="mk_sb")
        kp_sb = pers.tile([128, NKT], f32, name="kp_sb")
        bq_sb = pers.tile([128, NPAIR], f32, name="bq_sb")
        bk_sb = pers.tile([128, NPAIR], f32, name="bk_sb")
        sel2_sb = pers.tile([2, 128], f32, name="sel2_sb")
        vones_sb = pers.tile([128, NKT * HPG], bf16, name="vones_sb")

        # input DMA over 3 queues; the f-major startup sweep consumes, per f,
        # the [Q0|K0|V] weight slice and the first 1024 x columns -- split the
        # f=0 pieces so the first sweep matmuls can start ASAP, keep later f
        # pieces arriving in f order, defer the filler weights / upper x
        # columns / w_out
        sy, sc, gp = nc.sync, nc.scalar, nc.gpsimd
        sc.dma_start(vones_sb[:], vones_d[:])
        sc.dma_start(kp_sb[:], kp_d[:])
        sc.dma_start(bq_sb[:], bq_d[:])
        sc.dma_start(bk_sb[:], bk_d[:])
        sc.dma_start(sel2_sb[:], sel2_d[:])
        sy.dma_start(xT_sb[:, 0, 0:256], xT_d[0, :, 0:256])
        gp.dma_start(wk_sb[:, 0, 256:768], wqkvT_d[0, :, 256:768])
        sc.dma_start(wk_sb[:, 0, 0:256], wqkvT_d[0, :, 0:256])
        sy.dma_start(xT_sb[:, 0, 256:640], xT_d[0, :, 256:640])
        sc.dma_start(xT_sb[:, 0, 640:1024], xT_d[0, :, 640:1024])
        sy.dma_start(wk_sb[:, 1, 0:768], wqkvT_d[1, :, 0:768])
        gp.dma_start(xT_sb[:, 1, 0:1024], xT_d[1, :, 0:1024])
        sc.dma_start(mq_sb[:], mq_d[:])
        sc.dma_start(mk_sb[:], mk_d[:])
        wrot = [sy, gp, sc]
        for f in range(2, FCH):
            wrot[f % 3].dma_start(wk_sb[:, f, 0:768], wqkvT_d[f, :, 0:768])
            wrot[(f + 1) % 3].dma_start(xT_sb[:, f, 0:1024], xT_d[f, :, 0:1024])
        for f in range(FCH):
            wrot[f % 3].dma_start(wk_sb[:, f, 768:1536], wqkvT_d[f, :, 768:1536])
        for f in range(FCH):
            wrot[(f + 1) % 3].dma_start(xT_sb[:, f, 1024:2048], xT_d[f, :, 1024:2048])
        for cchunk in range(NPAIR):
            wrot[(cchunk + 2) % 3].dma_start(wo_sb[:, cchunk, :], woutT_d[cchunk])
        nc.vector.tensor_copy(
            Vt[:, :, :, HD],
            vones_sb[:].rearrange("p (a b) -> p a b", a=NKT))

        # host packs the weight columns as [Q0 | K0 | V | Q1-3 | K1-3] so the
        # startup sweep's needs are one contiguous priority DMA slice
        def wQ(f, p):
            return wk_sb[:, f, 0:128] if p == 0 else \
                wk_sb[:, f, 768 + 128 * (p - 1):768 + 128 * p]

        def wK(f, p):
            return wk_sb[:, f, 128:256] if p == 0 else \
                wk_sb[:, f, 1152 + 128 * (p - 1):1152 + 128 * p]

        def wV(f):
            return wk_sb[:, f, 256:768]

        def v_copy(pv, t):
            # key-padding folded in: padded key rows of V are zeroed (the
            # ones column comes pre-masked from the host via vones)
            nc.vector.tensor_scalar(
                Vt[:, t, :, 0:HD],
                pv[:].rearrange("p (h d) -> p h d", h=HPG),
                kp_sb[:, t:t + 1], None, MUL)

        def qk_store(pqk, tgt, t5):
            pair = tgt % 4
            dst = (QT if tgt < 4 else KT)[:, pair, t5 * 512:(t5 + 1) * 512]
            bias = (bq_sb if tgt < 4 else bk_sb)[:, pair:pair + 1]
            nc.vector.tensor_scalar(dst, pqk[:], bias, None, ADD)

        # ---- startup: f-major sweep (PE works while x^T still streams) ----
        with tc.tile_pool(name="psUp", bufs=1, space="PSUM") as psUp:
            pvs = [psUp.tile([128, 512], f32, name=f"pv{t}", tag=f"u{t}")
                   for t in range(5)]
            pk0 = psUp.tile([128, 512], f32, name="pk0", tag="u5")
            pk1 = psUp.tile([128, 512], f32, name="pk1", tag="u6")
            pq0 = psUp.tile([128, 512], f32, name="pq0", tag="u7")
            for f in range(FCH):
                st = dict(start=(f == 0), stop=(f == FCH - 1))
                for t in range(5):
                    nc.tensor.matmul(pvs[t][:], xT_sb[:, f, t * 128:(t + 1) * 128],
                                     wV(f), **st)
                nc.tensor.matmul(pk0[:], wK(f, 0), xT_sb[:, f, 0:512], **st)
                nc.tensor.matmul(pk1[:], wK(f, 0), xT_sb[:, f, 512:1024], **st)
                nc.tensor.matmul(pq0[:], wQ(f, 0), xT_sb[:, f, 0:512], **st)
            for t in range(5):
                v_copy(pvs[t], t)
            nc.vector.tensor_scalar(KT[:, 0, 0:512], pk0[:], bk_sb[:, 0:1], None, ADD)
            nc.vector.tensor_scalar(KT[:, 0, 512:1024], pk1[:], bk_sb[:, 0:1], None, ADD)
            nc.vector.tensor_scalar(QT[:, 0, 0:512], pq0[:], bq_sb[:, 0:1], None, ADD)

        # ---- attention with dependency-scheduled projection fillers ----
        with tc.tile_pool(name="eps", bufs=3) as epool, \
             tc.tile_pool(name="nsb", bufs=1) as nsb, \
             tc.tile_pool(name="avp", bufs=1) as avp, \
             tc.tile_pool(name="osb", bufs=2) as osb, \
             tc.tile_pool(name="psAv", bufs=1, space="PSUM") as psAv, \
             tc.tile_pool(name="psSc", bufs=2, space="PSUM") as psSc, \
             tc.tile_pool(name="psX", bufs=1, space="PSUM") as psX:

            nx = [0]

            def emitQK(tgt, t5):
                pqk = psX.tile([128, 512], f32, name="pqk", tag=f"x{nx[0] % 2}")
                nx[0] += 1
                w = wQ if tgt < 4 else wK
                for f in range(FCH):
                    nc.tensor.matmul(pqk[:], w(f, tgt % 4),
                                     xT_sb[:, f, t5 * 512:(t5 + 1) * 512],
                                     start=(f == 0), stop=(f == FCH - 1))
                qk_store(pqk, tgt, t5)

            def emitV(t):
                pv = psX.tile([128, 512], f32, name="pv", tag=f"x{nx[0] % 2}")
                nx[0] += 1
                for f in range(FCH):
                    nc.tensor.matmul(pv[:], xT_sb[:, f, t * 128:(t + 1) * 128],
                                     wV(f), start=(f == 0), stop=(f == FCH - 1))
                v_copy(pv, t)

            # filler units per (q5, pair): each QKV projection unit is placed
            # in the latest stretch that still finishes before its consumer
            # (K(p,t5) = (4+p, t5); Q(p,t5) = (p, t5); V per 128-key tile)
            F = {
                (0, 0): [(5, 0), (1, 0), (5, 1), (6, 0), (2, 0)],
                (0, 1): [(6, 1), (7, 0), (3, 0)],
                (0, 2): [(7, 1), (0, 1), (4, 2)],
                (0, 3): [("V", 5), ("V", 6), ("V", 7), ("V", 8)],
                (1, 0): [(1, 1), (5, 2)],
                (1, 1): [(2, 1), (6, 2)],
                (1, 2): [(3, 1), (7, 2)],
                (1, 3): [(0, 2), (4, 3), ("V", 9), ("V", 10)],
                (2, 0): [(1, 2), (5, 3)],
                (2, 1): [(2, 2), (6, 3)],
                (2, 2): [(3, 2), (7, 3)],
                (2, 3): [(0, 3)],
                (3, 0): [(1, 3)],
                (3, 1): [(2, 3)],
                (3, 2): [(3, 3)],
                (3, 3): [],
            }
            # V tiles consumed late within the same pair's kt loop must be
            # emitted mid-loop (before the consuming kt), not after it
            M = {
                (2, 0, 1): ("V", 11), (2, 0, 3): ("V", 12),
                (3, 0, 1): ("V", 13), (3, 0, 3): ("V", 14),
                (3, 0, 5): ("V", 15),
            }

            norm_pend = {}

            def emit_normalize_pair(q5, p):
                av2, rc32 = norm_pend.pop((q5, p))
                qsl = slice(q5 * 512, (q5 + 1) * 512)
                bc = psX.tile([128, 512], f32, name="bc", tag="x0")
                nc.tensor.matmul(bc[:], sel2_sb[:], rc32[:],
                                 start=True, stop=True)
                nc.vector.tensor_tensor(AT[0:64, p, qsl], av2[0:64, 0, :],
                                        bc[0:64, :], MUL)
                nc.vector.tensor_tensor(AT[64:128, p, qsl], av2[0:64, 1, :],
                                        bc[64:128, :], MUL)

            def emit_proj(q5):
                for tq in range(4):
                    t = 4 * q5 + tq
                    tsl = slice(t * 128, (t + 1) * 128)
                    po0 = psX.tile([128, 512], f32, name="po0", tag="x0")
                    po1 = psX.tile([128, 512], f32, name="po1", tag="x1")
                    for cchunk in range(NPAIR):
                        lhsT = AT[:, cchunk, tsl]
                        nc.tensor.matmul(po0[:], lhsT, wo_sb[:, cchunk, 0:512],
                                         start=(cchunk == 0), stop=(cchunk == 3))
                        nc.tensor.matmul(po1[:], lhsT, wo_sb[:, cchunk, 512:1024],
                                         start=(cchunk == 0), stop=(cchunk == 3))
                    ot = osb.tile([128, D], f32, name="ot", tag="ot")
                    nc.vector.tensor_copy(ot[:, 0:512], po0[:])
                    nc.vector.tensor_copy(ot[:, 512:1024], po1[:])
                    (nc.sync if t % 2 else nc.gpsimd).dma_start(
                        out_d[tsl, :], ot[:])

            # Flat software pipeline across pairs/blocks: each pair's last AV
            # is deferred past the next pair's first score group so the PE
            # never drains while ScalarE finishes the last exp.
            pend_av = [None]

            def flush_av():
                if pend_av[0] is not None:
                    pend_av[0]()
                    pend_av[0] = None

            def finish_pair(q5, p, avA, avB):
                # AV out of PSUM into one combined tile (slot-freeing copies
                # first), then both denominator rows gathered with a single
                # SBUF->SBUF DMA (DVE is partition-locked) for one batched
                # fp32 reciprocal
                av2 = avp.tile([HD + 1, 2, 512], f32,
                               name=f"av2_{p}", tag=f"avp{p}")
                nc.vector.tensor_copy(av2[:, 0, :], avA[:])
                nc.vector.tensor_copy(av2[:, 1, :], avB[:])
                d2 = nsb.tile([2, 512], f32, name="d2", tag=f"dp{p}")
                nc.gpsimd.dma_start(d2[:], av2[64:65, :, :])
                rc32 = nsb.tile([2, 512], f32, name="rc32", tag=f"di{p}")
                nc.vector.reciprocal_approx_fast(rc32[:], d2[:])
                norm_pend[(q5, p)] = (av2, rc32)

            for q5 in range(TQ5):
                nkt = min(4 * q5 + 5, NKT)
                q5s = q5 * 512
                for p in range(NPAIR):
                    avA = psAv.tile([HD + 1, 512], f32, name="avA", tag="avA")
                    avB = psAv.tile([HD + 1, 512], f32, name="avB", tag="avB")
                    for kt in range(nkt):
                        ks = slice(kt * 128, (kt + 1) * 128)
                        off = kt - 4 * q5
                        masked = off >= 0
                        # masked tiles only affect queries >= q0
                        q0, m1 = _mwin(off) if masked else (0, 512)
                        qsl = slice(q5s + q0, q5s + 512)
                        sc2 = psSc.tile([128, 2, 512], f32, name="sc2", tag="sc2")
                        nc.tensor.matmul(sc2[:, 0, q0:512],
                                         KT[0:64, p, ks], QT[0:64, p, qsl],
                                         start=True, stop=not masked,
                                         tile_position=(0, 0))
                        nc.tensor.matmul(sc2[:, 1, q0:512],
                                         KT[64:128, p, ks], QT[64:128, p, qsl],
                                         start=True, stop=not masked,
                                         tile_position=(64, 0))
                        if masked:
                            nc.tensor.matmul(sc2[:, 0, q0:m1], mk_sb[:],
                                             mq_sb[:, off, 0:m1 - q0],
                                             start=False, stop=True,
                                             skip_group_check=True)
                            nc.tensor.matmul(sc2[:, 1, q0:m1], mk_sb[:],
                                             mq_sb[:, off, 0:m1 - q0],
                                             start=False, stop=True,
                                             skip_group_check=True)
                        flush_av()
                        if kt == 1:
                            # deferred bookkeeping once the pipeline is primed:
                            # block-delayed normalize of (q5-1, p), plus
                            # pair-delayed normalize inside the last block
                            if q5 >= 1 and (q5 - 1, p) in norm_pend:
                                emit_normalize_pair(q5 - 1, p)
                                if p == 3:
                                    emit_proj(q5 - 1)
                        if kt == 2 and q5 == TQ5 - 1 and p >= 1 and \
                                (q5, p - 1) in norm_pend:
                            emit_normalize_pair(q5, p - 1)
                        e2 = epool.tile([128, 2, 512], bf16, name="e2", tag="e2")
                        nc.scalar.activation(e2[:, :, q0:512], sc2[:, :, q0:512],
                                             Exp, scale=1.0 / math.sqrt(HD))

                        def mk_av(kt=kt, e2=e2, q0=q0, avA=avA, avB=avB,
                                  p=p, nkt=nkt, q5=q5):
                            nc.tensor.matmul(avA[0:65, q0:512],
                                             Vt[:, kt, 2 * p, :],
                                             e2[:, 0, q0:512],
                                             start=(kt == 0), stop=(kt == nkt - 1),
                                             skip_group_check=True)
                            nc.tensor.matmul(avB[0:65, q0:512],
                                             Vt[:, kt, 2 * p + 1, :],
                                             e2[:, 1, q0:512],
                                             start=(kt == 0), stop=(kt == nkt - 1),
                                             skip_group_check=True)
                            if kt == nkt - 1:
                                finish_pair(q5, p, avA, avB)
                        pend_av[0] = mk_av
                        mid = M.get((q5, p, kt))
                        if mid is not None:
                            emitV(mid[1])
                    # dependency-scheduled QKV fillers keep the PE warm
                    for j, item in enumerate(F[(q5, p)]):
                        if item[0] == "V":
                            emitV(item[1])
                        else:
                            emitQK(item[0], item[1])
                        if j == 0:
                            flush_av()
            # drain: last pair's AV, its normalize, last projection
            flush_av()
            emit_normalize_pair(TQ5 - 1, 3)
            emit_proj(TQ5 - 1)

    nc.finalize()
    return nc


def _host_inputs(x, key_padding_mask, w_qkv, b_qkv, w_out):
    """Per-core input dicts."""
    import ml_dtypes

    f32 = np.float32
    bf = ml_dtypes.bfloat16
    # masks (shared across cores): mq holds only the 128-wide diagonal
    # window [q0, m1) per off
    k = np.arange(128)[:, None]
    mq = np.zeros((128, 5, 128), f32)
    for off in range(5):
        q0, m1 = _mwin(off)
        j = np.arange(m1 - q0)[None, :]
        mq[:, off, 0:m1 - q0] = (128 * off + k > q0 + j + L).astype(f32)
    mq = mq.astype(bf)
    mk = (NEG * np.eye(128, dtype=f32)).astype(bf)
    sel2 = np.zeros((2, 128), f32)
    sel2[0, 0:64] = 1.0
    sel2[1, 64:128] = 1.0

    in_maps = []
    for c in range(NCORES):
        b, g = divmod(c, 2)
        # channel rows for this group's Q/K (pairs of heads -> 128 rows each)
        qrows = np.concatenate(
            [w_qkv[64 * (8 * g + 2 * p):64 * (8 * g + 2 * p) + 128] for p in range(NPAIR)])
        krows = np.concatenate(
            [w_qkv[D + 64 * (8 * g + 2 * p):D + 64 * (8 * g + 2 * p) + 128] for p in range(NPAIR)])
        vrows = w_qkv[2 * D + 512 * g:2 * D + 512 * g + 512]
        # column order [Q0 | K0 | V | Q1-3 | K1-3]: the startup sweep's
        # weights form one contiguous priority DMA slice
        w_all = np.concatenate([qrows[0:128], krows[0:128], vrows,
                                qrows[128:512], krows[128:512]], 0)
        wqkvT = np.ascontiguousarray(w_all.T).reshape(FCH, 128, 3 * 512)
        bq = np.stack(
            [b_qkv[64 * (8 * g + 2 * p):64 * (8 * g + 2 * p) + 128] for p in range(NPAIR)], 1)
        bk = np.stack(
            [b_qkv[D + 64 * (8 * g + 2 * p):D + 64 * (8 * g + 2 * p) + 128] for p in range(NPAIR)], 1)
        xT = np.ascontiguousarray(x[b].T).reshape(FCH, 128, T)
        woutT = np.ascontiguousarray(w_out.T[512 * g:512 * g + 512]).reshape(NPAIR, 128, D)
        # key-padding as a 0/1 keep-multiplier on V rows + the ones column
        kp01 = np.ascontiguousarray(
            (1.0 - key_padding_mask[b].astype(f32)).reshape(NKT, 128).T)
        vones = np.repeat(kp01, HPG, axis=1).astype(bf)
        in_maps.append({
            "xT": xT.astype(bf), "wqkvT": wqkvT.astype(bf),
            "woutT": woutT.astype(bf),
            "bq": bq.astype(f32), "bk": bk.astype(f32),
            "kp01": kp01.astype(f32),
            "mq": mq, "mk": mk, "sel2": sel2.astype(f32), "vones": vones,
        })
    return in_maps


def kernel(x, key_padding_mask, w_qkv, b_qkv, w_out, b_out):
    from concourse.bass_utils import run_bass_kernel_spmd

    x = np.asarray(x, np.float32)
    key_padding_mask = np.asarray(key_padding_mask)
    w_qkv = np.asarray(w_qkv, np.float32)
    b_qkv = np.asarray(b_qkv, np.float32)
    w_out = np.asarray(w_out, np.float32)
    b_out = np.asarray(b_out, np.float32)

    if "nc" not in _BUILT:
        _BUILT["nc"] = _build_nc()
    nc = _BUILT["nc"]

    in_maps = _host_inputs(x, key_padding_mask, w_qkv, b_qkv, w_out)
    res = run_bass_kernel_spmd(nc, in_maps, core_ids=list(range(NCORES)))
    out = np.empty((B, T, D), np.float32)
    for b in range(B):
        out[b] = res.results[2 * b]["out_part"] + res.results[2 * b + 1]["out_part"]
    # host-folded biases: b_out plus the V-bias pushed through the projection
    bv = b_qkv[2 * D:3 * D]
    out += (b_out + bv @ w_out.T)[None, None, :].astype(np.float32)
    return out


# revision 8
# speedup vs baseline: 1.0774x; 1.0574x over previous
"""CT self-attention (causal + 2 future frames) for Trainium2, 8 NeuronCores.

Sharding: batch (4-way) x head-group (2-way): core c = 2*b + g handles batch b,
heads [8g, 8g+8). Each core computes its QKV projection slice, banded
attention for its 8 heads, and a partial output projection; the host sums the
two partial outputs per batch and adds the (host-folded) biases.

All matmuls run in bf16 (1 col/cycle @ 2.4 GHz warm) with fp32 PSUM.
Schedule is built for PE density (HAM stays at full clock) and balances the
PE against the exp-bound ScalarE stream:
  - startup: f-major sweep accumulating V t0-4 + K(0,0-1) + Q(0,0) in 8 PSUM
    banks while the x^T / w DMAs stream in (3 DMA queues, split first
    chunks); attention starts right after the sweep
  - the rest of the QKV projection (V t5-15, all other K/Q blocks) is
    distributed as PE gap filler across ALL attention blocks, placed by a
    dependency schedule (each unit lands just before the block that consumes
    it) so the PE never drains during the exp-bound late blocks
  - attention: scores S_T = K^T-tile.T @ Q (2 heads row-tiled), CT mask via
    accumulating -1e9*I @ MQ matmul (mq holds only the 128-wide diagonal
    window), exp on ScalarE (scale 1/8, no bias), AV with a ones column on V
    (denominator on partition 64); key-padding is folded into V rows and the
    ones column (both zeroed for padded keys), not the exp bias
  - normalize per head pair: 2 denominator rows -> [2,512] fast reciprocal
    (fp32), one [2,128] fp32 selector matmul broadcasts both heads'
    reciprocals across 128 partitions, 2 DVE mults into AT; deferred into
    the next block's stream (immediate for the last block)
  - output projection per 128-query tile once all 4 pairs are normalized
"""
import math
from contextlib import ExitStack

import numpy as np

B, T, D, H = 4, 2048, 1024, 16
HD = D // H            # 64
L = 2                  # max_future_frames
NCORES = 8
HPG = 8                # heads per group/core
NPAIR = 4              # head pairs per core
FCH = 8                # feature chunks (D / 128)
TQ5 = 4                # 512-wide query tiles
NKT = 16               # 128-wide key tiles
NEG = -1.0e9

_BUILT = {}


def _mwin(off):
    q0 = max(0, 128 * off - L)
    m1 = min(512, 128 * off + 126)
    return q0, m1


def _build_nc():
    import concourse.tile as tile
    from concourse import bacc, mybir

    dt = mybir.dt
    f32, bf16 = dt.float32, dt.bfloat16
    Exp = mybir.ActivationFunctionType.Exp
    MUL = mybir.AluOpType.mult
    ADD = mybir.AluOpType.add

    nc = bacc.Bacc(None, target_bir_lowering=False)
    xT_d = nc.dram_tensor("xT", [FCH, 128, T], bf16, kind="ExternalInput")
    wqkvT_d = nc.dram_tensor("wqkvT", [FCH, 128, 3 * 512], bf16, kind="ExternalInput")
    woutT_d = nc.dram_tensor("woutT", [NPAIR, 128, D], bf16, kind="ExternalInput")
    bq_d = nc.dram_tensor("bq", [128, NPAIR], f32, kind="ExternalInput")
    bk_d = nc.dram_tensor("bk", [128, NPAIR], f32, kind="ExternalInput")
    kp_d = nc.dram_tensor("kp01", [128, NKT], f32, kind="ExternalInput")
    mq_d = nc.dram_tensor("mq", [128, 5, 2, 128], bf16, kind="ExternalInput")
    mk_d = nc.dram_tensor("mk", [128, 128], bf16, kind="ExternalInput")
    sel2_d = nc.dram_tensor("sel2", [2, 128], bf16, kind="ExternalInput")
    vones_d = nc.dram_tensor("vones", [128, NKT * HPG], bf16, kind="ExternalInput")
    out_d = nc.dram_tensor("out_part", [T, D], bf16, kind="ExternalOutput")

    with tile.TileContext(nc) as tc, \
         nc.allow_low_precision(reason="bf16 matmul fast path"), \
         ExitStack() as top:
        pers = top.enter_context(tc.tile_pool(name="pers", bufs=1))
        xT_sb = pers.tile([128, FCH, T], bf16, name="xT_sb")
        wk_sb = pers.tile([128, FCH, 3 * 512], bf16, name="wk_sb")
        QT = pers.tile([128, NPAIR, T], bf16, name="QT")
        KT = pers.tile([128, NPAIR, T], bf16, name="KT")
        Vt = pers.tile([128, NKT, HPG, HD + 1], bf16, name="Vt")
        AT = pers.tile([128, NPAIR, T], bf16, name="AT")
        wo_sb = pers.tile([128, NPAIR, D], bf16, name="wo_sb")
        mq_sb = pers.tile([128, 5, 2, 128], bf16, name="mq_sb")
        mk_sb = pers.tile([128, 128], bf16, name="mk_sb")
        kp_sb = pers.tile([128, NKT], f32, name="kp_sb")
        bq_sb = pers.tile([128, NPAIR], f32, name="bq_sb")
        bk_sb = pers.tile([128, NPAIR], f32, name="bk_sb")
        sel2_sb = pers.tile([2, 128], bf16, name="sel2_sb")
        vones_sb = pers.tile([128, NKT * HPG], bf16, name="vones_sb")

        # input DMA over 3 queues; the f-major startup sweep consumes, per f,
        # the [Q0|K0|V] weight slice and the first 1024 x columns -- split the
        # f=0 pieces so the first sweep matmuls can start ASAP, keep later f
        # pieces arriving in f order, defer the filler weights / upper x
        # columns / w_out
        sy, sc, gp = nc.sync, nc.scalar, nc.gpsimd
        sc.dma_start(vones_sb[:], vones_d[:])
        sc.dma_start(kp_sb[:], kp_d[:])
        sc.dma_start(bq_sb[:], bq_d[:])
        sc.dma_start(bk_sb[:], bk_d[:])
        sc.dma_start(sel2_sb[:], sel2_d[:])
        sy.dma_start(xT_sb[:, 0, 0:256], xT_d[0, :, 0:256])
        gp.dma_start(wk_sb[:, 0, 256:768], wqkvT_d[0, :, 256:768])
        sc.dma_start(wk_sb[:, 0, 0:256], wqkvT_d[0, :, 0:256])
        sy.dma_start(xT_sb[:, 0, 256:640], xT_d[0, :, 256:640])
        sc.dma_start(xT_sb[:, 0, 640:1024], xT_d[0, :, 640:1024])
        sy.dma_start(wk_sb[:, 1, 0:768], wqkvT_d[1, :, 0:768])
        gp.dma_start(xT_sb[:, 1, 0:1024], xT_d[1, :, 0:1024])
        sc.dma_start(mq_sb[:], mq_d[:])
        sc.dma_start(mk_sb[:], mk_d[:])
        wrot = [sy, gp, sc]
        for f in range(2, FCH):
            wrot[f % 3].dma_start(wk_sb[:, f, 0:768], wqkvT_d[f, :, 0:768])
            wrot[(f + 1) % 3].dma_start(xT_sb[:, f, 0:1024], xT_d[f, :, 0:1024])
        for f in range(FCH):
            wrot[f % 3].dma_start(wk_sb[:, f, 768:1536], wqkvT_d[f, :, 768:1536])
        for f in range(FCH):
            wrot[(f + 1) % 3].dma_start(xT_sb[:, f, 1024:2048], xT_d[f, :, 1024:2048])
        for cchunk in range(NPAIR):
            wrot[(cchunk + 2) % 3].dma_start(wo_sb[:, cchunk, :], woutT_d[cchunk])
        nc.vector.tensor_copy(
            Vt[:, :, :, HD],
            vones_sb[:].rearrange("p (a b) -> p a b", a=NKT))

        # host packs the weight columns as [Q0 | K0 | V | Q1-3 | K1-3] so the
        # startup sweep's needs are one contiguous priority DMA slice
        def wQ(f, p):
            return wk_sb[:, f, 0:128] if p == 0 else \
                wk_sb[:, f, 768 + 128 * (p - 1):768 + 128 * p]

        def wK(f, p):
            return wk_sb[:, f, 128:256] if p == 0 else \
                wk_sb[:, f, 1152 + 128 * (p - 1):1152 + 128 * p]

        def wV(f):
            return wk_sb[:, f, 256:768]

        def v_copy(pv, t):
            # key-padding folded in: padded key rows of V are zeroed (the
            # ones column comes pre-masked from the host via vones)
            nc.vector.tensor_scalar(
                Vt[:, t, :, 0:HD],
                pv[:].rearrange("p (h d) -> p h d", h=HPG),
                kp_sb[:, t:t + 1], None, MUL)

        def qk_store(pqk, tgt, t5):
            pair = tgt % 4
            dst = (QT if tgt < 4 else KT)[:, pair, t5 * 512:(t5 + 1) * 512]
            bias = (bq_sb if tgt < 4 else bk_sb)[:, pair:pair + 1]
            nc.vector.tensor_scalar(dst, pqk[:], bias, None, ADD)

        # ---- startup: f-major sweep (PE works while x^T still streams) ----
        with tc.tile_pool(name="psUp", bufs=1, space="PSUM") as psUp:
            pvs = [psUp.tile([128, 512], f32, name=f"pv{t}", tag=f"u{t}")
                   for t in range(5)]
            pk0 = psUp.tile([128, 512], f32, name="pk0", tag="u5")
            pk1 = psUp.tile([128, 512], f32, name="pk1", tag="u6")
            pq0 = psUp.tile([128, 512], f32, name="pq0", tag="u7")
            for f in range(FCH):
                st = dict(start=(f == 0), stop=(f == FCH - 1))
                for t in range(5):
                    nc.tensor.matmul(pvs[t][:], xT_sb[:, f, t * 128:(t + 1) * 128],
                                     wV(f), **st)
                nc.tensor.matmul(pk0[:], wK(f, 0), xT_sb[:, f, 0:512], **st)
                nc.tensor.matmul(pk1[:], wK(f, 0), xT_sb[:, f, 512:1024], **st)
                nc.tensor.matmul(pq0[:], wQ(f, 0), xT_sb[:, f, 0:512], **st)
            for t in range(5):
                v_copy(pvs[t], t)
            nc.vector.tensor_scalar(KT[:, 0, 0:512], pk0[:], bk_sb[:, 0:1], None, ADD)
            nc.vector.tensor_scalar(KT[:, 0, 512:1024], pk1[:], bk_sb[:, 0:1], None, ADD)
            nc.vector.tensor_scalar(QT[:, 0, 0:512], pq0[:], bq_sb[:, 0:1], None, ADD)

        # ---- attention with dependency-scheduled projection fillers ----
        with tc.tile_pool(name="eps", bufs=3) as epool, \
             tc.tile_pool(name="nsb", bufs=1) as nsb, \
             tc.tile_pool(name="avp", bufs=1) as avp, \
             tc.tile_pool(name="osb", bufs=2) as osb, \
             tc.tile_pool(name="psAv", bufs=1, space="PSUM") as psAv, \
             tc.tile_pool(name="psSc", bufs=2, space="PSUM") as psSc, \
             tc.tile_pool(name="psX", bufs=1, space="PSUM") as psX:

            nx = [0]

            def emitQK(tgt, t5):
                pqk = psX.tile([128, 512], f32, name="pqk", tag=f"x{nx[0] % 2}")
                nx[0] += 1
                w = wQ if tgt < 4 else wK
                for f in range(FCH):
                    nc.tensor.matmul(pqk[:], w(f, tgt % 4),
                                     xT_sb[:, f, t5 * 512:(t5 + 1) * 512],
                                     start=(f == 0), stop=(f == FCH - 1))
                qk_store(pqk, tgt, t5)

            def emitV(t):
                pv = psX.tile([128, 512], f32, name="pv", tag=f"x{nx[0] % 2}")
                nx[0] += 1
                for f in range(FCH):
                    nc.tensor.matmul(pv[:], xT_sb[:, f, t * 128:(t + 1) * 128],
                                     wV(f), start=(f == 0), stop=(f == FCH - 1))
                v_copy(pv, t)

            # filler units per (q5, pair): each QKV projection unit is placed
            # in the latest stretch that still finishes before its consumer
            # (K(p,t5) = (4+p, t5); Q(p,t5) = (p, t5); V per 128-key tile)
            F = {
                (0, 0): [(5, 0), (1, 0), (5, 1), (6, 0), (2, 0)],
                (0, 1): [(6, 1), (7, 0), (3, 0)],
                (0, 2): [(7, 1), (0, 1), (4, 2)],
                (0, 3): [("V", 5), ("V", 6), ("V", 7), ("V", 8)],
                (1, 0): [(1, 1), (5, 2)],
                (1, 1): [(2, 1), (6, 2)],
                (1, 2): [(3, 1), (7, 2)],
                (1, 3): [(0, 2), (4, 3), ("V", 9), ("V", 10)],
                (2, 0): [(1, 2), (5, 3)],
                (2, 1): [(2, 2), (6, 3)],
                (2, 2): [(3, 2), (7, 3)],
                (2, 3): [(0, 3)],
                (3, 0): [(1, 3)],
                (3, 1): [(2, 3)],
                (3, 2): [(3, 3)],
                (3, 3): [],
            }
            # V tiles consumed late within the same pair's kt loop must be
            # emitted mid-loop (before the consuming kt), not after it
            M = {
                (2, 0, 1): ("V", 11), (2, 0, 3): ("V", 12),
                (3, 0, 1): ("V", 13), (3, 0, 3): ("V", 14),
                (3, 0, 5): ("V", 15),
            }

            norm_pend = {}

            def emit_normalize_pair(q5, p):
                av2, recp = norm_pend.pop((q5, p))
                qsl = slice(q5 * 512, (q5 + 1) * 512)
                bc = psX.tile([128, 512], f32, name="bc", tag="x0")
                nc.tensor.matmul(bc[:], sel2_sb[:], recp[:],
                                 start=True, stop=True)
                nc.vector.tensor_tensor(AT[0:64, p, qsl], av2[0:64, 0, :],
                                        bc[0:64, :], MUL)
                nc.vector.tensor_tensor(AT[64:128, p, qsl], av2[0:64, 1, :],
                                        bc[64:128, :], MUL)

            def emit_proj(q5):
                for tq in range(4):
                    t = 4 * q5 + tq
                    tsl = slice(t * 128, (t + 1) * 128)
                    po0 = psX.tile([128, 512], f32, name="po0", tag="x0")
                    po1 = psX.tile([128, 512], f32, name="po1", tag="x1")
                    for cchunk in range(NPAIR):
                        lhsT = AT[:, cchunk, tsl]
                        nc.tensor.matmul(po0[:], lhsT, wo_sb[:, cchunk, 0:512],
                                         start=(cchunk == 0), stop=(cchunk == 3))
                        nc.tensor.matmul(po1[:], lhsT, wo_sb[:, cchunk, 512:1024],
                                         start=(cchunk == 0), stop=(cchunk == 3))
                    ot = osb.tile([128, D], bf16, name="ot", tag="ot")
                    nc.vector.tensor_copy(ot[:, 0:512], po0[:])
                    nc.vector.tensor_copy(ot[:, 512:1024], po1[:])
                    (nc.sync if t % 2 else nc.gpsimd).dma_start(
                        out_d[tsl, :], ot[:])

            # Flat software pipeline across pairs/blocks: each pair's last AV
            # is deferred past the next pair's first score group so the PE
            # never drains while ScalarE finishes the last exp.
            pend_av = [None]

            def flush_av():
                if pend_av[0] is not None:
                    pend_av[0]()
                    pend_av[0] = None

            def finish_pair(q5, p, avA, avB):
                # AV out of PSUM into one combined tile (slot-freeing copies
                # first), then both denominator rows gathered with a single
                # SBUF->SBUF DMA (DVE is partition-locked) for one batched
                # fp32 reciprocal
                av2 = avp.tile([HD + 1, 2, 512], f32,
                               name=f"av2_{p}", tag=f"avp{p}")
                nc.vector.tensor_copy(av2[:, 0, :], avA[:])
                nc.vector.tensor_copy(av2[:, 1, :], avB[:])
                d2 = nsb.tile([2, 512], f32, name="d2", tag=f"dp{p}")
                nc.gpsimd.dma_start(d2[:], av2[64:65, :, :])
                rc32 = nsb.tile([2, 512], f32, name="rc32", tag=f"di{p}")
                nc.vector.reciprocal_approx_fast(rc32[:], d2[:])
                recp = nsb.tile([2, 512], bf16, name="recp", tag=f"rc{p}")
                nc.vector.tensor_copy(recp[:], rc32[:])
                norm_pend[(q5, p)] = (av2, recp)

            for q5 in range(TQ5):
                nkt = min(4 * q5 + 5, NKT)
                q5s = q5 * 512
                for p in range(NPAIR):
                    avA = psAv.tile([HD + 1, 512], f32, name="avA", tag="avA")
                    avB = psAv.tile([HD + 1, 512], f32, name="avB", tag="avB")
                    for kt in range(nkt):
                        ks = slice(kt * 128, (kt + 1) * 128)
                        off = kt - 4 * q5
                        masked = off >= 0
                        # masked tiles only affect queries >= q0
                        q0, m1 = _mwin(off) if masked else (0, 512)
                        qsl = slice(q5s + q0, q5s + 512)
                        sc2 = psSc.tile([128, 2, 512], f32, name="sc2", tag="sc2")
                        nc.tensor.matmul(sc2[:, 0, q0:512],
                                         KT[0:64, p, ks], QT[0:64, p, qsl],
                                         start=True, stop=not masked,
                                         tile_position=(0, 0))
                        nc.tensor.matmul(sc2[:, 1, q0:512],
                                         KT[64:128, p, ks], QT[64:128, p, qsl],
                                         start=True, stop=not masked,
                                         tile_position=(64, 0))
                        if masked:
                            nc.tensor.matmul(sc2[:, 0:2, q0:m1], mk_sb[:],
                                             mq_sb[:, off, :, 0:m1 - q0],
                                             start=False, stop=True,
                                             skip_group_check=True)
                        flush_av()
                        if kt == 1:
                            # deferred bookkeeping once the pipeline is primed:
                            # block-delayed normalize of (q5-1, p), plus
                            # pair-delayed normalize inside the last block
                            if q5 >= 1 and (q5 - 1, p) in norm_pend:
                                emit_normalize_pair(q5 - 1, p)
                                if p == 3:
                                    emit_proj(q5 - 1)
                        if kt == 2 and q5 == TQ5 - 1 and p >= 1 and \
                                (q5, p - 1) in norm_pend:
                            emit_normalize_pair(q5, p - 1)
                        e2 = epool.tile([128, 2, 512], bf16, name="e2", tag="e2")
                        nc.scalar.activation(e2[:, :, q0:512], sc2[:, :, q0:512],
                                             Exp, scale=1.0 / math.sqrt(HD))

                        def mk_av(kt=kt, e2=e2, q0=q0, avA=avA, avB=avB,
                                  p=p, nkt=nkt, q5=q5):
                            nc.tensor.matmul(avA[0:65, q0:512],
                                             Vt[:, kt, 2 * p, :],
                                             e2[:, 0, q0:512],
                                             start=(kt == 0), stop=(kt == nkt - 1),
                                             skip_group_check=True)
                            nc.tensor.matmul(avB[0:65, q0:512],
                                             Vt[:, kt, 2 * p + 1, :],
                                             e2[:, 1, q0:512],
                                             start=(kt == 0), stop=(kt == nkt - 1),
                                             skip_group_check=True)
                            if kt == nkt - 1:
                                finish_pair(q5, p, avA, avB)
                        pend_av[0] = mk_av
                        mid = M.get((q5, p, kt))
                        if mid is not None:
                            emitV(mid[1])
                    # dependency-scheduled QKV fillers keep the PE warm
                    for j, item in enumerate(F[(q5, p)]):
                        if item[0] == "V":
                            emitV(item[1])
                        else:
                            emitQK(item[0], item[1])
                        if j == 0:
                            flush_av()
            # drain: last pair's AV, its normalize, last projection
            flush_av()
            emit_normalize_pair(TQ5 - 1, 3)
            emit_proj(TQ5 - 1)

    nc.finalize()
    return nc


def _host_inputs(x, key_padding_mask, w_qkv, b_qkv, w_out):
    """Per-core input dicts."""
    import ml_dtypes

    f32 = np.float32
    bf = ml_dtypes.bfloat16
    # masks (shared across cores): mq holds only the 128-wide diagonal
    # window [q0, m1) per off
    k = np.arange(128)[:, None]
    mq = np.zeros((128, 5, 2, 128), f32)
    for off in range(5):
        q0, m1 = _mwin(off)
        j = np.arange(m1 - q0)[None, :]
        msk = (128 * off + k > q0 + j + L).astype(f32)
        mq[:, off, 0, 0:m1 - q0] = msk
        mq[:, off, 1, 0:m1 - q0] = msk
    mq = mq.astype(bf)
    mk = (NEG * np.eye(128, dtype=f32)).astype(bf)
    sel2 = np.zeros((2, 128), f32)
    sel2[0, 0:64] = 1.0
    sel2[1, 64:128] = 1.0

    in_maps = []
    for c in range(NCORES):
        b, g = divmod(c, 2)
        # channel rows for this group's Q/K (pairs of heads -> 128 rows each)
        qrows = np.concatenate(
            [w_qkv[64 * (8 * g + 2 * p):64 * (8 * g + 2 * p) + 128] for p in range(NPAIR)])
        krows = np.concatenate(
            [w_qkv[D + 64 * (8 * g + 2 * p):D + 64 * (8 * g + 2 * p) + 128] for p in range(NPAIR)])
        vrows = w_qkv[2 * D + 512 * g:2 * D + 512 * g + 512]
        # column order [Q0 | K0 | V | Q1-3 | K1-3]: the startup sweep's
        # weights form one contiguous priority DMA slice
        w_all = np.concatenate([qrows[0:128], krows[0:128], vrows,
                                qrows[128:512], krows[128:512]], 0)
        wqkvT = np.ascontiguousarray(w_all.T).reshape(FCH, 128, 3 * 512)
        bq = np.stack(
            [b_qkv[64 * (8 * g + 2 * p):64 * (8 * g + 2 * p) + 128] for p in range(NPAIR)], 1)
        bk = np.stack(
            [b_qkv[D + 64 * (8 * g + 2 * p):D + 64 * (8 * g + 2 * p) + 128] for p in range(NPAIR)], 1)
        xT = np.ascontiguousarray(x[b].T).reshape(FCH, 128, T)
        woutT = np.ascontiguousarray(w_out.T[512 * g:512 * g + 512]).reshape(NPAIR, 128, D)
        # key-padding as a 0/1 keep-multiplier on V rows + the ones column
        kp01 = np.ascontiguousarray(
            (1.0 - key_padding_mask[b].astype(f32)).reshape(NKT, 128).T)
        vones = np.repeat(kp01, HPG, axis=1).astype(bf)
        in_maps.append({
            "xT": xT.astype(bf), "wqkvT": wqkvT.astype(bf),
            "woutT": woutT.astype(bf),
            "bq": bq.astype(f32), "bk": bk.astype(f32),
            "kp01": kp01.astype(f32),
            "mq": mq, "mk": mk, "sel2": sel2.astype(bf), "vones": vones,
        })
    return in_maps


def kernel(x, key_padding_mask, w_qkv, b_qkv, w_out, b_out):
    from concourse.bass_utils import run_bass_kernel_spmd

    x = np.asarray(x, np.float32)
    key_padding_mask = np.asarray(key_padding_mask)
    w_qkv = np.asarray(w_qkv, np.float32)
    b_qkv = np.asarray(b_qkv, np.float32)
    w_out = np.asarray(w_out, np.float32)
    b_out = np.asarray(b_out, np.float32)

    if "nc" not in _BUILT:
        _BUILT["nc"] = _build_nc()
    nc = _BUILT["nc"]

    in_maps = _host_inputs(x, key_padding_mask, w_qkv, b_qkv, w_out)
    res = run_bass_kernel_spmd(nc, in_maps, core_ids=list(range(NCORES)))
    out = np.empty((B, T, D), np.float32)
    for b in range(B):
        out[b] = (res.results[2 * b]["out_part"].astype(np.float32)
                  + res.results[2 * b + 1]["out_part"].astype(np.float32))
    # host-folded biases: b_out plus the V-bias pushed through the projection
    bv = b_qkv[2 * D:3 * D]
    out += (b_out + bv @ w_out.T)[None, None, :].astype(np.float32)
    return out


# revision 10
# speedup vs baseline: 1.1217x; 1.0411x over previous
"""CT self-attention (causal + 2 future frames) for Trainium2, 8 NeuronCores.

Sharding: batch (4-way) x head-group (2-way): core c = 2*b + g handles batch b,
heads [8g, 8g+8). Each core computes its QKV projection slice, banded
attention for its 8 heads, and a partial output projection; the host sums the
two partial outputs per batch and adds the (host-folded) biases.

All matmuls run in bf16 (1 col/cycle @ 2.4 GHz warm) with fp32 PSUM.
Schedule is built for PE density (HAM stays at full clock) and balances the
PE against the exp-bound ScalarE stream:
  - startup: f-major sweep accumulating V t0-4 + K(0,0-1) + Q(0,0) in 8 PSUM
    banks while the x^T / w DMAs stream in (3 DMA queues, split first
    chunks); attention starts right after the sweep
  - the rest of the QKV projection (V t5-15, all other K/Q blocks) is
    distributed as PE gap filler across ALL attention blocks, placed by a
    dependency schedule (each unit lands just before the block that consumes
    it) so the PE never drains during the exp-bound late blocks
  - attention: scores S_T = K^T-tile.T @ Q (2 heads row-tiled), CT mask via
    accumulating -1e9*I @ MQ matmul (mq holds only the 128-wide diagonal
    window), exp on ScalarE (scale 1/8, no bias), AV with a ones column on V
    (denominator on partition 64); key-padding is folded into V rows and the
    ones column (both zeroed for padded keys), not the exp bias
  - normalize per head pair: 2 denominator rows -> [2,512] fast reciprocal
    (fp32), one [2,128] fp32 selector matmul broadcasts both heads'
    reciprocals across 128 partitions, 2 DVE mults into AT; deferred into
    the next block's stream (immediate for the last block)
  - output projection per 128-query tile once all 4 pairs are normalized
"""
import math
from contextlib import ExitStack

import numpy as np

B, T, D, H = 4, 2048, 1024, 16
HD = D // H            # 64
L = 2                  # max_future_frames
NCORES = 8
HPG = 8                # heads per group/core
NPAIR = 4              # head pairs per core
FCH = 8                # feature chunks (D / 128)
TQ5 = 4                # 512-wide query tiles
NKT = 16               # 128-wide key tiles
NEG = -1.0e9

_BUILT = {}


def _mwin(off):
    q0 = max(0, 128 * off - L)
    m1 = min(512, 128 * off + 126)
    return q0, m1


def _build_nc():
    import concourse.tile as tile
    from concourse import bacc, mybir

    dt = mybir.dt
    f32, bf16 = dt.float32, dt.bfloat16
    Exp = mybir.ActivationFunctionType.Exp
    MUL = mybir.AluOpType.mult
    ADD = mybir.AluOpType.add

    nc = bacc.Bacc(None, target_bir_lowering=False)
    xT_d = nc.dram_tensor("xT", [FCH, 128, T], bf16, kind="ExternalInput")
    wqkvT_d = nc.dram_tensor("wqkvT", [FCH, 128, 3 * 512], bf16, kind="ExternalInput")
    woutT_d = nc.dram_tensor("woutT", [NPAIR, 128, D], bf16, kind="ExternalInput")
    bq_d = nc.dram_tensor("bq", [128, NPAIR], f32, kind="ExternalInput")
    bk_d = nc.dram_tensor("bk", [128, NPAIR], f32, kind="ExternalInput")
    kp_d = nc.dram_tensor("kp01", [128, NKT], f32, kind="ExternalInput")
    vm_d = nc.dram_tensor("vm", [128, 5, 2, 128], bf16, kind="ExternalInput")
    sel2_d = nc.dram_tensor("sel2", [2, 128], bf16, kind="ExternalInput")
    vones_d = nc.dram_tensor("vones", [128, NKT * HPG], bf16, kind="ExternalInput")
    out_d = nc.dram_tensor("out_part", [T, D], bf16, kind="ExternalOutput")

    with tile.TileContext(nc) as tc, \
         nc.allow_low_precision(reason="bf16 matmul fast path"), \
         ExitStack() as top:
        pers = top.enter_context(tc.tile_pool(name="pers", bufs=1))
        xT_sb = pers.tile([128, FCH, T], bf16, name="xT_sb")
        wk_sb = pers.tile([128, FCH, 3 * 512], bf16, name="wk_sb")
        QT = pers.tile([128, NPAIR, T], bf16, name="QT")
        KT = pers.tile([128, NPAIR, T], bf16, name="KT")
        Vt = pers.tile([128, NKT, HPG, HD + 1], bf16, name="Vt")
        AT = pers.tile([128, NPAIR, T], bf16, name="AT")
        wo_sb = pers.tile([128, NPAIR, D], bf16, name="wo_sb")
        vm_sb = pers.tile([128, 5, 2, 128], bf16, name="vm_sb")
        kp_sb = pers.tile([128, NKT], f32, name="kp_sb")
        bq_sb = pers.tile([128, NPAIR], f32, name="bq_sb")
        bk_sb = pers.tile([128, NPAIR], f32, name="bk_sb")
        sel2_sb = pers.tile([2, 128], bf16, name="sel2_sb")
        vones_sb = pers.tile([128, NKT * HPG], bf16, name="vones_sb")

        # input DMA over 3 queues; the f-major startup sweep consumes, per f,
        # the [Q0|K0|V] weight slice and the first 1024 x columns -- split the
        # f=0 pieces so the first sweep matmuls can start ASAP, keep later f
        # pieces arriving in f order, defer the filler weights / upper x
        # columns / w_out
        sy, sc, gp = nc.sync, nc.scalar, nc.gpsimd
        sc.dma_start(vones_sb[:], vones_d[:])
        sc.dma_start(kp_sb[:], kp_d[:])
        sc.dma_start(bq_sb[:], bq_d[:])
        sc.dma_start(bk_sb[:], bk_d[:])
        sc.dma_start(sel2_sb[:], sel2_d[:])
        sy.dma_start(xT_sb[:, 0, 0:256], xT_d[0, :, 0:256])
        gp.dma_start(wk_sb[:, 0, 256:768], wqkvT_d[0, :, 256:768])
        sc.dma_start(wk_sb[:, 0, 0:256], wqkvT_d[0, :, 0:256])
        sy.dma_start(xT_sb[:, 0, 256:640], xT_d[0, :, 256:640])
        sc.dma_start(xT_sb[:, 0, 640:1024], xT_d[0, :, 640:1024])
        sy.dma_start(wk_sb[:, 1, 0:768], wqkvT_d[1, :, 0:768])
        gp.dma_start(xT_sb[:, 1, 0:1024], xT_d[1, :, 0:1024])
        sc.dma_start(vm_sb[:], vm_d[:])
        wrot = [sy, gp, sc]
        for f in range(2, FCH):
            wrot[f % 3].dma_start(wk_sb[:, f, 0:768], wqkvT_d[f, :, 0:768])
            wrot[(f + 1) % 3].dma_start(xT_sb[:, f, 0:1024], xT_d[f, :, 0:1024])
        # pair-1 filler weights (Q1/K1 column slices) land first so block-0
        # fillers are never DMA-starved, then the bulk of the filler weights
        for f in range(FCH):
            wrot[f % 3].dma_start(wk_sb[:, f, 768:896], wqkvT_d[f, :, 768:896])
            wrot[(f + 1) % 3].dma_start(wk_sb[:, f, 1152:1280],
                                        wqkvT_d[f, :, 1152:1280])
        for f in range(FCH):
            wrot[f % 3].dma_start(wk_sb[:, f, 896:1152], wqkvT_d[f, :, 896:1152])
            wrot[(f + 1) % 3].dma_start(wk_sb[:, f, 1280:1536],
                                        wqkvT_d[f, :, 1280:1536])
        for f in range(FCH):
            wrot[(f + 1) % 3].dma_start(xT_sb[:, f, 1024:2048], xT_d[f, :, 1024:2048])
        for cchunk in range(NPAIR):
            wrot[(cchunk + 2) % 3].dma_start(wo_sb[:, cchunk, :], woutT_d[cchunk])
        nc.vector.tensor_copy(
            Vt[:, :, :, HD],
            vones_sb[:].rearrange("p (a b) -> p a b", a=NKT))

        # host packs the weight columns as [Q0 | K0 | V | Q1-3 | K1-3] so the
        # startup sweep's needs are one contiguous priority DMA slice
        def wQ(f, p):
            return wk_sb[:, f, 0:128] if p == 0 else \
                wk_sb[:, f, 768 + 128 * (p - 1):768 + 128 * p]

        def wK(f, p):
            return wk_sb[:, f, 128:256] if p == 0 else \
                wk_sb[:, f, 1152 + 128 * (p - 1):1152 + 128 * p]

        def wV(f):
            return wk_sb[:, f, 256:768]

        def v_copy(pv, t):
            # key-padding folded in: padded key rows of V are zeroed (the
            # ones column comes pre-masked from the host via vones)
            nc.vector.tensor_scalar(
                Vt[:, t, :, 0:HD],
                pv[:].rearrange("p (h d) -> p h d", h=HPG),
                kp_sb[:, t:t + 1], None, MUL)

        def qk_store(pqk, tgt, t5):
            pair = tgt % 4
            dst = (QT if tgt < 4 else KT)[:, pair, t5 * 512:(t5 + 1) * 512]
            bias = (bq_sb if tgt < 4 else bk_sb)[:, pair:pair + 1]
            nc.vector.tensor_scalar(dst, pqk[:], bias, None, ADD)

        # ---- startup: f-major sweep (PE works while x^T still streams) ----
        with tc.tile_pool(name="psUp", bufs=1, space="PSUM") as psUp:
            pvs = [psUp.tile([128, 512], f32, name=f"pv{t}", tag=f"u{t}")
                   for t in range(5)]
            pk0 = psUp.tile([128, 512], f32, name="pk0", tag="u5")
            pk1 = psUp.tile([128, 512], f32, name="pk1", tag="u6")
            pq0 = psUp.tile([128, 512], f32, name="pq0", tag="u7")
            for f in range(FCH):
                st = dict(start=(f == 0), stop=(f == FCH - 1))
                for t in range(5):
                    nc.tensor.matmul(pvs[t][:], xT_sb[:, f, t * 128:(t + 1) * 128],
                                     wV(f), **st)
                nc.tensor.matmul(pk0[:], wK(f, 0), xT_sb[:, f, 0:512], **st)
                nc.tensor.matmul(pk1[:], wK(f, 0), xT_sb[:, f, 512:1024], **st)
                nc.tensor.matmul(pq0[:], wQ(f, 0), xT_sb[:, f, 0:512], **st)
            for t in range(5):
                v_copy(pvs[t], t)
            nc.vector.tensor_scalar(KT[:, 0, 0:512], pk0[:], bk_sb[:, 0:1], None, ADD)
            nc.vector.tensor_scalar(KT[:, 0, 512:1024], pk1[:], bk_sb[:, 0:1], None, ADD)
            nc.vector.tensor_scalar(QT[:, 0, 0:512], pq0[:], bq_sb[:, 0:1], None, ADD)

        # ---- attention with dependency-scheduled projection fillers ----
        with tc.tile_pool(name="eps", bufs=3) as epool, \
             tc.tile_pool(name="nsb", bufs=1) as nsb, \
             tc.tile_pool(name="avp", bufs=1) as avp, \
             tc.tile_pool(name="osb", bufs=2) as osb, \
             tc.tile_pool(name="psAv", bufs=1, space="PSUM") as psAv, \
             tc.tile_pool(name="psSc", bufs=2, space="PSUM") as psSc, \
             tc.tile_pool(name="psX", bufs=1, space="PSUM") as psX:

            nx = [0]

            def emitQK(tgt, t5):
                pqk = psX.tile([128, 512], f32, name="pqk", tag=f"x{nx[0] % 2}")
                nx[0] += 1
                w = wQ if tgt < 4 else wK
                for f in range(FCH):
                    nc.tensor.matmul(pqk[:], w(f, tgt % 4),
                                     xT_sb[:, f, t5 * 512:(t5 + 1) * 512],
                                     start=(f == 0), stop=(f == FCH - 1))
                qk_store(pqk, tgt, t5)

            def emitV(t):
                pv = psX.tile([128, 512], f32, name="pv", tag=f"x{nx[0] % 2}")
                nx[0] += 1
                for f in range(FCH):
                    nc.tensor.matmul(pv[:], xT_sb[:, f, t * 128:(t + 1) * 128],
                                     wV(f), start=(f == 0), stop=(f == FCH - 1))
                v_copy(pv, t)

            # filler units per (q5, pair): each QKV projection unit is placed
            # in the latest stretch that still finishes before its consumer
            # (K(p,t5) = (4+p, t5); Q(p,t5) = (p, t5); V per 128-key tile)
            F = {
                (0, 0): [(5, 0), (1, 0), (5, 1), (6, 0), (2, 0)],
                (0, 1): [(6, 1), (7, 0), (3, 0)],
                (0, 2): [(7, 1), (0, 1), (4, 2)],
                (0, 3): [("V", 5), ("V", 6), ("V", 7), ("V", 8)],
                (1, 0): [(1, 1), (5, 2)],
                (1, 1): [(2, 1), (6, 2)],
                (1, 2): [(3, 1), (7, 2)],
                (1, 3): [(0, 2), (4, 3), ("V", 9), ("V", 10)],
                (2, 0): [(1, 2), (5, 3)],
                (2, 1): [(2, 2), (6, 3)],
                (2, 2): [(3, 2), (7, 3)],
                (2, 3): [(0, 3)],
                (3, 0): [(1, 3)],
                (3, 1): [(2, 3)],
                (3, 2): [(3, 3)],
                (3, 3): [],
            }
            # V tiles consumed late within the same pair's kt loop must be
            # emitted mid-loop (before the consuming kt), not after it
            M = {
                (2, 0, 1): ("V", 11), (2, 0, 3): ("V", 12),
                (3, 0, 1): ("V", 13), (3, 0, 3): ("V", 14),
                (3, 0, 5): ("V", 15),
            }

            norm_pend = {}

            def emit_normalize_pair(q5, p):
                av2, recp = norm_pend.pop((q5, p))
                qsl = slice(q5 * 512, (q5 + 1) * 512)
                bc = psX.tile([128, 512], f32, name="bc", tag="x0")
                nc.tensor.matmul(bc[:], sel2_sb[:], recp[:],
                                 start=True, stop=True)
                nc.vector.tensor_tensor(AT[0:64, p, qsl], av2[0:64, 0, :],
                                        bc[0:64, :], MUL)
                nc.vector.tensor_tensor(AT[64:128, p, qsl], av2[0:64, 1, :],
                                        bc[64:128, :], MUL)

            def emit_proj(q5):
                for tq in range(4):
                    t = 4 * q5 + tq
                    tsl = slice(t * 128, (t + 1) * 128)
                    po0 = psX.tile([128, 512], f32, name="po0", tag="x0")
                    po1 = psX.tile([128, 512], f32, name="po1", tag="x1")
                    for cchunk in range(NPAIR):
                        lhsT = AT[:, cchunk, tsl]
                        nc.tensor.matmul(po0[:], lhsT, wo_sb[:, cchunk, 0:512],
                                         start=(cchunk == 0), stop=(cchunk == 3))
                        nc.tensor.matmul(po1[:], lhsT, wo_sb[:, cchunk, 512:1024],
                                         start=(cchunk == 0), stop=(cchunk == 3))
                    ot = osb.tile([128, D], bf16, name="ot", tag="ot")
                    nc.vector.tensor_copy(ot[:, 0:512], po0[:])
                    nc.vector.tensor_copy(ot[:, 512:1024], po1[:])
                    (nc.sync if t % 2 else nc.gpsimd).dma_start(
                        out_d[tsl, :], ot[:])

            # Flat software pipeline across pairs/blocks: each pair's last AV
            # is deferred past the next pair's first score group so the PE
            # never drains while ScalarE finishes the last exp.
            pend_av = [None]

            def flush_av():
                if pend_av[0] is not None:
                    pend_av[0]()
                    pend_av[0] = None

            def finish_pair(q5, p, avA, avB):
                # AV out of PSUM into one combined tile (slot-freeing copies
                # first), then both denominator rows gathered with a single
                # SBUF->SBUF DMA (DVE is partition-locked) for one batched
                # fp32 reciprocal
                av2 = avp.tile([HD + 1, 2, 512], f32,
                               name=f"av2_{p}", tag=f"avp{p}")
                nc.vector.tensor_copy(av2[:, 0, :], avA[:])
                nc.vector.tensor_copy(av2[:, 1, :], avB[:])
                d2 = nsb.tile([2, 512], f32, name="d2", tag=f"dp{p}")
                nc.gpsimd.dma_start(d2[:], av2[64:65, :, :])
                rc32 = nsb.tile([2, 512], f32, name="rc32", tag=f"di{p}")
                nc.vector.reciprocal_approx_fast(rc32[:], d2[:])
                recp = nsb.tile([2, 512], bf16, name="recp", tag=f"rc{p}")
                nc.vector.tensor_copy(recp[:], rc32[:])
                norm_pend[(q5, p)] = (av2, recp)

            for q5 in range(TQ5):
                nkt = min(4 * q5 + 5, NKT)
                q5s = q5 * 512
                for p in range(NPAIR):
                    avA = psAv.tile([HD + 1, 512], f32, name="avA", tag="avA")
                    avB = psAv.tile([HD + 1, 512], f32, name="avB", tag="avB")
                    for kt in range(nkt):
                        ks = slice(kt * 128, (kt + 1) * 128)
                        off = kt - 4 * q5
                        masked = off >= 0
                        # masked tiles only affect queries >= q0
                        q0, m1 = _mwin(off) if masked else (0, 512)
                        qsl = slice(q5s + q0, q5s + 512)
                        sc2 = psSc.tile([128, 2, 512], f32, name="sc2", tag="sc2")
                        nc.tensor.matmul(sc2[:, 0, q0:512],
                                         KT[0:64, p, ks], QT[0:64, p, qsl],
                                         start=True, stop=True,
                                         tile_position=(0, 0))
                        nc.tensor.matmul(sc2[:, 1, q0:512],
                                         KT[64:128, p, ks], QT[64:128, p, qsl],
                                         start=True, stop=True,
                                         tile_position=(64, 0))
                        flush_av()
                        if kt == 1:
                            # deferred bookkeeping once the pipeline is primed:
                            # block-delayed normalize of (q5-1, p), plus
                            # pair-delayed normalize inside the last block
                            if q5 >= 1 and (q5 - 1, p) in norm_pend:
                                emit_normalize_pair(q5 - 1, p)
                                if p == 3:
                                    emit_proj(q5 - 1)
                        if kt == 2 and q5 == TQ5 - 1 and p >= 1 and \
                                (q5, p - 1) in norm_pend:
                            emit_normalize_pair(q5, p - 1)
                        e2 = epool.tile([128, 2, 512], bf16, name="e2", tag="e2")
                        nc.scalar.activation(e2[:, :, q0:512], sc2[:, :, q0:512],
                                             Exp, scale=1.0 / math.sqrt(HD))
                        if masked:
                            # CT mask post-exp: zero the invalid triangle of
                            # the diagonal window with a 0/1 DVE multiply
                            nc.vector.tensor_tensor(
                                e2[:, :, q0:m1], e2[:, :, q0:m1],
                                vm_sb[:, off, :, 0:m1 - q0], MUL)

                        def mk_av(kt=kt, e2=e2, q0=q0, m1=m1, masked=masked,
                                  avA=avA, avB=avB, p=p, nkt=nkt, q5=q5):
                            # masked tiles split: the fully-valid tail
                            # [m1:512] runs straight off the exp; only the
                            # small triangle [q0:m1] waits for the DVE mask
                            parts = ([(m1, 512), (q0, m1)] if masked
                                     else [(q0, 512)])
                            emitted = [pt for pt in parts if pt[0] < pt[1]]
                            for i, (a, b_) in enumerate(emitted):
                                st = (kt == 0) and i == 0
                                sp = (kt == nkt - 1) and i == len(emitted) - 1
                                nc.tensor.matmul(avA[0:65, a:b_],
                                                 Vt[:, kt, 2 * p, :],
                                                 e2[:, 0, a:b_],
                                                 start=st, stop=sp,
                                                 skip_group_check=True)
                                nc.tensor.matmul(avB[0:65, a:b_],
                                                 Vt[:, kt, 2 * p + 1, :],
                                                 e2[:, 1, a:b_],
                                                 start=st, stop=sp,
                                                 skip_group_check=True)
                            if kt == nkt - 1:
                                finish_pair(q5, p, avA, avB)
                        pend_av[0] = mk_av
                        mid = M.get((q5, p, kt))
                        if mid is not None:
                            emitV(mid[1])
                    # dependency-scheduled QKV fillers keep the PE warm
                    for j, item in enumerate(F[(q5, p)]):
                        if item[0] == "V":
                            emitV(item[1])
                        else:
                            emitQK(item[0], item[1])
                        if j == 0:
                            flush_av()
            # drain: last pair's AV, its normalize, last projection
            flush_av()
            emit_normalize_pair(TQ5 - 1, 3)
            emit_proj(TQ5 - 1)

    nc.finalize()
    return nc


def _host_inputs(x, key_padding_mask, w_qkv, b_qkv, w_out):
    """Per-core input dicts."""
    import ml_dtypes

    f32 = np.float32
    bf = ml_dtypes.bfloat16
    # masks (shared across cores): mq holds only the 128-wide diagonal
    # window [q0, m1) per off
    k = np.arange(128)[:, None]
    vm = np.ones((128, 5, 2, 128), f32)
    for off in range(5):
        q0, m1 = _mwin(off)
        j = np.arange(m1 - q0)[None, :]
        valid = (128 * off + k <= q0 + j + L).astype(f32)
        vm[:, off, 0, 0:m1 - q0] = valid
        vm[:, off, 1, 0:m1 - q0] = valid
    vm = vm.astype(bf)
    sel2 = np.zeros((2, 128), f32)
    sel2[0, 0:64] = 1.0
    sel2[1, 64:128] = 1.0

    in_maps = []
    for c in range(NCORES):
        b, g = divmod(c, 2)
        # channel rows for this group's Q/K (pairs of heads -> 128 rows each)
        qrows = np.concatenate(
            [w_qkv[64 * (8 * g + 2 * p):64 * (8 * g + 2 * p) + 128] for p in range(NPAIR)])
        krows = np.concatenate(
            [w_qkv[D + 64 * (8 * g + 2 * p):D + 64 * (8 * g + 2 * p) + 128] for p in range(NPAIR)])
        vrows = w_qkv[2 * D + 512 * g:2 * D + 512 * g + 512]
        # column order [Q0 | K0 | V | Q1-3 | K1-3]: the startup sweep's
        # weights form one contiguous priority DMA slice
        w_all = np.concatenate([qrows[0:128], krows[0:128], vrows,
                                qrows[128:512], krows[128:512]], 0)
        wqkvT = np.ascontiguousarray(w_all.T).reshape(FCH, 128, 3 * 512)
        bq = np.stack(
            [b_qkv[64 * (8 * g + 2 * p):64 * (8 * g + 2 * p) + 128] for p in range(NPAIR)], 1)
        bk = np.stack(
            [b_qkv[D + 64 * (8 * g + 2 * p):D + 64 * (8 * g + 2 * p) + 128] for p in range(NPAIR)], 1)
        xT = np.ascontiguousarray(x[b].T).reshape(FCH, 128, T)
        woutT = np.ascontiguousarray(w_out.T[512 * g:512 * g + 512]).reshape(NPAIR, 128, D)
        # key-padding as a 0/1 keep-multiplier on V rows + the ones column
        kp01 = np.ascontiguousarray(
            (1.0 - key_padding_mask[b].astype(f32)).reshape(NKT, 128).T)
        vones = np.repeat(kp01, HPG, axis=1).astype(bf)
        in_maps.append({
            "xT": xT.astype(bf), "wqkvT": wqkvT.astype(bf),
            "woutT": woutT.astype(bf),
            "bq": bq.astype(f32), "bk": bk.astype(f32),
            "kp01": kp01.astype(f32),
            "vm": vm, "sel2": sel2.astype(bf), "vones": vones,
        })
    return in_maps


def kernel(x, key_padding_mask, w_qkv, b_qkv, w_out, b_out):
    from concourse.bass_utils import run_bass_kernel_spmd

    x = np.asarray(x, np.float32)
    key_padding_mask = np.asarray(key_padding_mask)
    w_qkv = np.asarray(w_qkv, np.float32)
    b_qkv = np.asarray(b_qkv, np.float32)
    w_out = np.asarray(w_out, np.float32)
    b_out = np.asarray(b_out, np.float32)

    if "nc" not in _BUILT:
        _BUILT["nc"] = _build_nc()
    nc = _BUILT["nc"]

    in_maps = _host_inputs(x, key_padding_mask, w_qkv, b_qkv, w_out)
    res = run_bass_kernel_spmd(nc, in_maps, core_ids=list(range(NCORES)))
    out = np.empty((B, T, D), np.float32)
    for b in range(B):
        out[b] = (res.results[2 * b]["out_part"].astype(np.float32)
                  + res.results[2 * b + 1]["out_part"].astype(np.float32))
    # host-folded biases: b_out plus the V-bias pushed through the projection
    bv = b_qkv[2 * D:3 * D]
    out += (b_out + bv @ w_out.T)[None, None, :].astype(np.float32)
    return out
